# revision 1
# baseline (speedup 1.0000x reference)
"""Causal FFT-conv (B=32, Cin=Cout=128, L=K=4096) for 8 trn2 NeuronCores.

Strategy: host computes rFFTs (N=8192) of padded x and conj-rFFT of the
weight; the dominant frequency-domain channel contraction
  yhat[f, b, o] = sum_c xhat[f, c, b] * ghat[f, c, o]   (complex)
runs on-device as fp32 matmuls, sharded over frequency across the 8
cores (each frequency bin is independent).  Host then does the inverse
rFFT, crops to L, and adds bias.
"""

import sys

sys.path.insert(0, "/opt/trn_rl_repo")

import numpy as np

B, C, O, L, K = 32, 128, 128, 4096, 4096
N = 8192            # linear-conv FFT length (L + (K+1) - 1 with odd-padded kernel)
F = N // 2 + 1      # 4097 rfft bins
NCORES = 8
last_exec_ns = None
_nc_cache = None
FC = 513            # per-core frequency bins (8*513 = 4104 >= 4097, zero padded)
NFB = 19            # f-block per inner loop iteration; 27 blocks of 19 = 513
assert NFB * 27 == FC


def _build_bass():
    from concourse import bass, bacc, mybir
    from concourse.tile import TileContext

    dt = mybir.dt.float32
    dtb = mybir.dt.bfloat16
    nc = bacc.Bacc(None, target_bir_lowering=False)
    # Per-core inputs, frequency-major so the c-contraction is the partition dim.
    # per-f row layout along free dim: [ReX(32) | ImX(32) | -ImX(32) | ReW(128) | ImW(128)]
    pk = nc.dram_tensor("pk", [FC, C, 3 * B + 2 * O], dtb, kind="ExternalInput")
    y = nc.dram_tensor("y", [FC, 2, B, O], dt, kind="ExternalOutput")

    with TileContext(nc) as tc:
        with (
            tc.tile_pool(name="xin", bufs=3) as xpool,
            tc.tile_pool(name="yout", bufs=6) as ypool,
            tc.tile_pool(name="ps", bufs=4, space="PSUM") as pspool,
        ):
            for blk in range(FC // NFB):
                f0 = blk * NFB
                W = 3 * B + 2 * O
                xt = xpool.tile([C, NFB * W], dtb, tag="x")
                nc.gpsimd.dma_start(out=xt.rearrange("c (f z) -> c f z", f=NFB), in_=pk[f0 : f0 + NFB].rearrange("f c z -> c f z"))
                # 27 bins -> 7 psum-bank groups of <=4 bins (4*128 fp32 = 1 bank)
                for g0 in range(0, NFB, 4):
                    gn = min(4, NFB - g0)
                    yr = pspool.tile([B, gn * O], dt, tag="yr")
                    yi = pspool.tile([B, gn * O], dt, tag="yi")
                    def ops(fi):
                        xb = fi * W
                        A = xt[:, xb : xb + B]
                        Bt = xt[:, xb + B : xb + 2 * B]
                        Bn = xt[:, xb + 2 * B : xb + 3 * B]
                        Cc = xt[:, xb + 3 * B : xb + 3 * B + O]
                        Dd = xt[:, xb + 3 * B + O : xb + 3 * B + 2 * O]
                        return A, Bt, Bn, Cc, Dd
                    # one accumulation group per PSUM bank (waits stay small)
                    for j in range(gn):
                        A, Bt, Bn, Cc, Dd = ops(g0 + j)
                        o_sl = slice(j * O, (j + 1) * O)
                        nc.tensor.matmul(yr[:, o_sl], A, Cc, start=(j == 0), stop=False)
                        nc.tensor.matmul(yr[:, o_sl], Bn, Dd, start=False, stop=(j == gn - 1))
                    for j in range(gn):
                        A, Bt, Bn, Cc, Dd = ops(g0 + j)
                        o_sl = slice(j * O, (j + 1) * O)
                        nc.tensor.matmul(yi[:, o_sl], A, Dd, start=(j == 0), stop=False)
                        nc.tensor.matmul(yi[:, o_sl], Bt, Cc, start=False, stop=(j == gn - 1))
                    yrs = ypool.tile([B, gn * O], dt, tag="yrs")
                    yis = ypool.tile([B, gn * O], dt, tag="yis")
                    nc.vector.tensor_copy(yrs, yr)
                    nc.vector.tensor_copy(yis, yi)
                    nc.gpsimd.dma_start(
                        out=y[f0 + g0 : f0 + g0 + gn, 0].rearrange("f b o -> b f o"),
                        in_=yrs.rearrange("b (f o) -> b f o", f=gn),
                    )
                    nc.gpsimd.dma_start(
                        out=y[f0 + g0 : f0 + g0 + gn, 1].rearrange("f b o -> b f o"),
                        in_=yis.rearrange("b (f o) -> b f o", f=gn),
                    )
    nc.compile()
    return nc


def kernel(x: np.ndarray, weight: np.ndarray, bias: np.ndarray) -> np.ndarray:
    from concourse.bass_utils import run_bass_kernel_spmd

    x = np.asarray(x, np.float32)
    weight = np.asarray(weight, np.float32)
    bias = np.asarray(bias, np.float32)

    # Host FFTs (match reference: odd-pad kernel left by 1, causal left-pad x).
    xp = np.pad(x, ((0, 0), (0, 0), (4096, 0)))                  # [B, C, 8192]
    wp = np.pad(weight, ((0, 0), (0, 0), (1, 0)))                # [O, C, 4097]
    xf = np.fft.rfft(xp, axis=-1)                                # [B, C, F]
    gf = np.conj(np.fft.rfft(wp, n=N, axis=-1))                  # [O, C, F]

    # Pad F -> 8*FC and reshape to per-core frequency slices, f-major.
    FP = NCORES * FC
    xfp = np.zeros((B, C, FP), np.complex64)
    xfp[:, :, :F] = xf
    gfp = np.zeros((O, C, FP), np.complex64)
    gfp[:, :, :F] = gf
    xfp = np.ascontiguousarray(xfp.transpose(2, 1, 0))           # [FP, C, B]
    gfp = np.ascontiguousarray(gfp.transpose(2, 1, 0))           # [FP, C, O]

    in_maps = []
    for r in range(NCORES):
        sl = slice(r * FC, (r + 1) * FC)
        xs, gs = xfp[sl], gfp[sl]
        pk = np.concatenate(
            [xs.real, xs.imag, -xs.imag, gs.real, gs.imag], axis=2
        )  # [FC, C, 3B+2O]
        import ml_dtypes
        in_maps.append({"pk": np.ascontiguousarray(pk).astype(ml_dtypes.bfloat16)})

    global _nc_cache
    if _nc_cache is None:
        _nc_cache = _build_bass()
    nc = _nc_cache
    res = run_bass_kernel_spmd(nc, in_maps, list(range(NCORES)))
    global last_exec_ns
    last_exec_ns = getattr(res, "exec_time_ns", None)
    ys = [res.results[r]["y"] for r in range(NCORES)]            # [FC, 2, B, O]
    yall = np.concatenate(ys, axis=0)[:F]                        # [F, 2, B, O]
    yf = (yall[:, 0] + 1j * yall[:, 1]).transpose(1, 2, 0)       # [B, O, F]
    out = np.fft.irfft(yf, n=N, axis=-1)[:, :, :L].astype(np.float32)
    return out + bias[None, :, None].astype(np.float32)



# revision 7
# speedup vs baseline: 5.0037x; 5.0037x over previous
"""Causal FFT-conv (B=32, Cin=Cout=128, L=K=4096) on 8 trn2 NeuronCores.

Pipeline (wire-byte minimized; the axon tunnel runs ~60-80 MB/s so
transfer dominates):
  host: rfft(x, 8192), rfft(w, 8192) via scipy (float32-preserving);
        slice spectra by frequency across the 8 cores, cast to bf16,
        upload asynchronously in chunks so FFT/cast overlaps transfer.
  device (per core, 513 bins): for each bin f the complex channel
        contraction  yhat[b,o] = sum_c xhat[c,b] * conj(what)[c,o]
        as TWO bf16 matmuls into one PSUM tile [32, 256] = (yr | yi):
          mm1: stat A=ReX [c,32], stream [Q|Pn] -> (A.Q | -A.P)
          mm2: stat B=ImX [c,32], stream [P|Q]  -> (B.P |  B.Q)
        with P=Im(W), Q=Re(W), Pn=-P (negated on device).  The padding
        shift twiddles (x left-pad 4096, w left-pad 1) are folded into
        a per-bin factor t[f] applied on host to the returned spectrum.
  host: assemble yhat, *= t, bias into bin 0, irfft, crop to L.
"""

import sys
import time

sys.path.insert(0, "/opt/trn_rl_repo")

import numpy as np

B, C, O, L = 32, 128, 128, 4096
N = 8192
F = N // 2 + 1          # 4097 rfft bins
NCORES = 8
FC = 513                # bins per core (8*513 = 4104 >= 4097, zero padded)
NFB = 19                # bins per inner block; 27 * 19 = 513
WIN = 448               # SBUF cols per bin: [A 32 | B 32 | P 128 | Q 128 | Pn 128]
NOCH = 4                # o-chunks for the weight spectrum upload (32 o each)

last_exec_ns = None
_cache = {}


def _build_bass():
    from concourse import bass, bacc, mybir
    from concourse.tile import TileContext

    dt = mybir.dt
    nc = bacc.Bacc(None, target_bir_lowering=False)

    xre = nc.dram_tensor("xre", [B, C, FC], dt.bfloat16, kind="ExternalInput")
    xim = nc.dram_tensor("xim", [B, C, FC], dt.bfloat16, kind="ExternalInput")
    wim = [
        nc.dram_tensor(f"wim{k}", [O // NOCH, C, FC], dt.bfloat16, kind="ExternalInput")
        for k in range(NOCH)
    ]
    wre = [
        nc.dram_tensor(f"wre{k}", [O // NOCH, C, FC], dt.bfloat16, kind="ExternalInput")
        for k in range(NOCH)
    ]
    y = nc.dram_tensor("y", [B, 2 * O, FC], dt.bfloat16, kind="ExternalOutput")

    OC = O // NOCH  # 32
    with TileContext(nc) as tc:
        with (
            tc.tile_pool(name="xin", bufs=2) as xpool,
            tc.tile_pool(name="stg", bufs=2) as spool,
            tc.tile_pool(name="yout", bufs=3) as ypool,
            tc.tile_pool(name="ps", bufs=4, space="PSUM") as pspool,
        ):
            for blk in range(FC // NFB):
                f0 = blk * NFB
                fsl = slice(f0, f0 + NFB)
                # Stage DRAM->SBUF keeping contiguous f-runs innermost
                # (DMA needs a shared contiguous final dim, <=3 dims); the
                # transpose into per-bin windows happens on vector engine.
                xst = spool.tile([C, 2 * B * NFB], dt.bfloat16, tag="xst")
                xsr = xst.rearrange("c (b f) -> c b f", f=NFB)
                nc.gpsimd.dma_start(
                    out=xsr[:, 0:B], in_=xre[:, :, fsl].rearrange("b c f -> c b f")
                )
                nc.gpsimd.dma_start(
                    out=xsr[:, B : 2 * B],
                    in_=xim[:, :, fsl].rearrange("b c f -> c b f"),
                )
                wstp = spool.tile([C, O * NFB], dt.bfloat16, tag="wstp")
                wstq = spool.tile([C, O * NFB], dt.bfloat16, tag="wstq")
                wpr = wstp.rearrange("c (o f) -> c o f", f=NFB)
                wqr = wstq.rearrange("c (o f) -> c o f", f=NFB)
                for k in range(NOCH):
                    osl = slice(OC * k, OC * (k + 1))
                    nc.gpsimd.dma_start(
                        out=wpr[:, osl],
                        in_=wim[k][:, :, fsl].rearrange("o c f -> c o f"),
                    )
                    nc.gpsimd.dma_start(
                        out=wqr[:, osl],
                        in_=wre[k][:, :, fsl].rearrange("o c f -> c o f"),
                    )

                xt = xpool.tile([C, NFB * WIN], dt.bfloat16, tag="x")
                xtr = xt.rearrange("c (f z) -> c f z", f=NFB)
                # window per bin: [A 0:32 | B 32:64 | P 64:192 | Q 192:320 | Pn 320:448]
                nc.vector.tensor_copy(
                    xtr[:, :, 0 : 2 * B], xst.rearrange("c (b f) -> c f b", f=NFB)
                )
                nc.vector.tensor_copy(
                    xtr[:, :, 64:192], wstp.rearrange("c (o f) -> c f o", f=NFB)
                )
                nc.vector.tensor_copy(
                    xtr[:, :, 192:320], wstq.rearrange("c (o f) -> c f o", f=NFB)
                )
                nc.vector.tensor_scalar_mul(
                    xtr[:, :, 320:448],
                    wstp.rearrange("c (o f) -> c f o", f=NFB),
                    -1.0,
                )

                # yo col = z*NFB + f  (z = r*O + o), so f stays contiguous
                yo = ypool.tile([B, 2 * O * NFB], dt.bfloat16, tag="yo")
                yor = yo.rearrange("b (z f) -> b z f", f=NFB)
                for p in range(0, NFB, 2):
                    gn = min(2, NFB - p)
                    ps = pspool.tile([B, gn * 256], dt.float32, tag="ps")
                    for j in range(gn):
                        wb = (p + j) * WIN
                        sl = slice(j * 256, (j + 1) * 256)
                        # (A.Q | -A.P) + (B.P | B.Q) -> (yr | yi)
                        nc.tensor.matmul(
                            ps[:, sl], xt[:, wb : wb + 32],
                            xt[:, wb + 192 : wb + 448], start=True, stop=False,
                        )
                        nc.tensor.matmul(
                            ps[:, sl], xt[:, wb + 32 : wb + 64],
                            xt[:, wb + 64 : wb + 320], start=False, stop=True,
                        )
                    nc.vector.tensor_copy(
                        yor[:, :, p : p + gn],
                        ps.rearrange("b (f z) -> b z f", f=gn),
                    )
                nc.gpsimd.dma_start(
                    out=y[:, :, fsl], in_=yor
                )
    nc.compile()
    return nc


def _make_runner(nc):
    """Vendored from bass2jax.run_bass_via_pjrt: same custom-call path, but
    accepts pre-committed sharded device arrays (so uploads overlap host
    work) and returns the device output array without blocking."""
    import jax
    from jax.sharding import Mesh, PartitionSpec
    from jax.experimental.shard_map import shard_map
    from concourse import bass2jax, mybir

    bass2jax.install_neuronx_cc_hook()

    partition_name = nc.partition_id_tensor.name if nc.partition_id_tensor else None
    in_names, out_names, out_avals, out_shapes = [], [], [], []
    for alloc in nc.m.functions[0].allocations:
        if type(alloc).__name__ != "MemoryLocationSet":
            continue
        name = alloc.memorylocations[0].name
        if alloc.kind == "ExternalInput":
            if name != partition_name:
                in_names.append(name)
        elif alloc.kind == "ExternalOutput":
            shape = tuple(alloc.tensor_shape)
            dtype = mybir.dt.np(alloc.dtype)
            out_names.append(name)
            out_avals.append(jax.core.ShapedArray(shape, dtype))
            out_shapes.append((shape, dtype))
    n_params = len(in_names)
    all_names = in_names + out_names
    if partition_name is not None:
        all_names = all_names + [partition_name]
    donate = tuple(range(n_params, n_params + len(out_names)))

    def _body(*args):
        operands = list(args)
        if partition_name is not None:
            operands.append(bass2jax.partition_id_tensor())
        outs = bass2jax._bass_exec_p.bind(
            *operands,
            out_avals=tuple(out_avals),
            in_names=tuple(all_names),
            out_names=tuple(out_names),
            lowering_input_output_aliases=(),
            sim_require_finite=True,
            sim_require_nnan=True,
            nc=nc,
        )
        return tuple(outs)

    devices = jax.devices()[:NCORES]
    mesh = Mesh(np.asarray(devices), ("core",))
    nargs = n_params + len(out_names)
    sharded = jax.jit(
        shard_map(
            _body,
            mesh=mesh,
            in_specs=(PartitionSpec("core"),) * nargs,
            out_specs=(PartitionSpec("core"),) * len(out_names),
            check_rep=False,
        ),
        donate_argnums=donate,
        keep_unused=True,
    )
    return sharded, in_names, out_names, out_shapes, mesh


def kernel(x: np.ndarray, weight: np.ndarray, bias: np.ndarray) -> np.ndarray:
    import ml_dtypes
    import scipy.fft as sf
    import jax
    from jax.sharding import NamedSharding, PartitionSpec

    bf16 = ml_dtypes.bfloat16
    x = np.asarray(x, np.float32)
    weight = np.asarray(weight, np.float32)
    bias = np.asarray(bias, np.float32)

    if "mesh" not in _cache:
        devices = jax.devices()[:NCORES]
        from jax.sharding import Mesh

        _cache["mesh"] = Mesh(np.asarray(devices), ("core",))
    sharding = NamedSharding(_cache["mesh"], PartitionSpec("core"))

    puts = {}

    def put(name, arr):
        puts[name] = jax.device_put(arr, sharding)

    # --- weight spectrum, chunked over o so cast overlaps upload ---
    OC = O // NOCH
    for k in range(NOCH):
        wf = sf.rfft(weight[k * OC : (k + 1) * OC], n=N, axis=-1)  # [32,C,F] c64
        gim = np.zeros((NCORES * OC, C, FC), bf16)
        gre = np.zeros((NCORES * OC, C, FC), bf16)
        for r in range(NCORES):
            sl = slice(r * FC, min((r + 1) * FC, F))
            n = sl.stop - sl.start
            gim[OC * r : OC * r + OC, :, :n] = wf.imag[:, :, sl].astype(bf16)
            gre[OC * r : OC * r + OC, :, :n] = wf.real[:, :, sl].astype(bf16)
        put(f"wim{k}", gim)
        put(f"wre{k}", gre)

    # --- x spectrum ---
    xf = sf.rfft(x, n=N, axis=-1)  # [B,C,F] c64
    gxr = np.zeros((NCORES * B, C, FC), bf16)
    gxi = np.zeros((NCORES * B, C, FC), bf16)
    for r in range(NCORES):
        sl = slice(r * FC, min((r + 1) * FC, F))
        n = sl.stop - sl.start
        gxr[B * r : B * r + B, :, :n] = xf.real[:, :, sl].astype(bf16)
        gxi[B * r : B * r + B, :, :n] = xf.imag[:, :, sl].astype(bf16)
    put("xre", gxr)
    put("xim", gxi)

    # --- build/compile while uploads stream ---
    if "nc" not in _cache:
        _cache["nc"] = _build_bass()
        _cache["runner"] = _make_runner(_cache["nc"])
    sharded, in_names, out_names, out_shapes, mesh = _cache["runner"]

    # donated zero output buffers (bf16 zeros compress well on the tunnel)
    zero_args = []
    for (shape, dtype), name in zip(out_shapes, out_names):
        z = np.zeros((NCORES * shape[0], *shape[1:]), dtype)
        zero_args.append(jax.device_put(z, NamedSharding(mesh, PartitionSpec("core"))))

    args = [puts[name] for name in in_names] + zero_args
    out_arrs = sharded(*args)
    yg = np.asarray(out_arrs[0])  # [8*B, 2, O, FC] bf16

    # --- host: assemble spectrum, twiddle, bias, inverse rfft ---
    ygf = yg.astype(np.float32).reshape(NCORES, B, 2, O, FC)
    yc = np.empty((B, O, NCORES * FC), np.complex64)
    for r in range(NCORES):
        yc.real[:, :, FC * r : FC * (r + 1)] = ygf[r, :, 0]
        yc.imag[:, :, FC * r : FC * (r + 1)] = ygf[r, :, 1]
    yv = yc[:, :, :F]
    # fold out the causal left-pad shifts: x by K-1=4096 -> (-1)^f, w by 1
    tw = np.exp(1j * np.pi * np.arange(F) * (N // 2 + 1) / (N // 2)).astype(
        np.complex64
    )
    yv *= tw
    yv[:, :, 0] += (bias * np.float32(N)).astype(np.float32)[None, :]
    out = sf.irfft(yv, n=N, axis=-1)[:, :, :L]
    return np.ascontiguousarray(out, dtype=np.float32)


# revision 9
# speedup vs baseline: 5.3141x; 1.0621x over previous
"""Causal FFT-conv (B=32, Cin=Cout=128, L=K=4096) on 8 trn2 NeuronCores.

Pipeline (wire-byte minimized; the axon tunnel runs ~60-80 MB/s so
transfer dominates):
  host: rfft(x, 8192), rfft(w, 8192) via scipy (float32-preserving);
        slice spectra by frequency across the 8 cores, cast to bf16,
        upload asynchronously in chunks so FFT/cast overlaps transfer.
  device (per core, 513 bins): for each bin f the complex channel
        contraction  yhat[b,o] = sum_c xhat[c,b] * conj(what)[c,o]
        as TWO bf16 matmuls into one PSUM tile [32, 256] = (yr | yi):
          mm1: stat A=ReX [c,32], stream [Q|Pn] -> (A.Q | -A.P)
          mm2: stat B=ImX [c,32], stream [P|Q]  -> (B.P |  B.Q)
        with P=Im(W), Q=Re(W), Pn=-P (negated on device).  The padding
        shift twiddles (x left-pad 4096, w left-pad 1) are folded into
        a per-bin factor t[f] applied on host to the returned spectrum.
  host: assemble yhat, *= t, bias into bin 0, irfft, crop to L.
"""

import sys
import time

sys.path.insert(0, "/opt/trn_rl_repo")

import numpy as np

B, C, O, L = 32, 128, 128, 4096
N = 8192
F = N // 2 + 1          # 4097 rfft bins
NCORES = 8
FC = 513                # bins per core (8*513 = 4104 >= 4097, zero padded)
NFB = 19                # bins per inner block; 27 * 19 = 513
WIN = 448               # SBUF cols per bin: [A 32 | B 32 | P 128 | Q 128 | Pn 128]
NOCH = 4                # o-chunks for the weight spectrum upload (32 o each)

last_exec_ns = None
_cache = {}


def _build_bass():
    from concourse import bass, bacc, mybir
    from concourse.tile import TileContext

    dt = mybir.dt
    nc = bacc.Bacc(None, target_bir_lowering=False)

    xc = nc.dram_tensor("xc", [B, 2, C, FC], dt.bfloat16, kind="ExternalInput")
    wc = [
        nc.dram_tensor(
            f"wc{k}", [O // NOCH, 2, C, FC], dt.bfloat16, kind="ExternalInput"
        )
        for k in range(NOCH)
    ]
    xre, xim = xc[:, 0], xc[:, 1]
    wim = [wc[k][:, 0] for k in range(NOCH)]
    wre = [wc[k][:, 1] for k in range(NOCH)]
    y = nc.dram_tensor("y", [B, 2 * O, FC], dt.bfloat16, kind="ExternalOutput")

    OC = O // NOCH  # 32
    with TileContext(nc) as tc:
        with (
            tc.tile_pool(name="xin", bufs=2) as xpool,
            tc.tile_pool(name="stg", bufs=2) as spool,
            tc.tile_pool(name="yout", bufs=3) as ypool,
            tc.tile_pool(name="ps", bufs=4, space="PSUM") as pspool,
        ):
            for blk in range(FC // NFB):
                f0 = blk * NFB
                fsl = slice(f0, f0 + NFB)
                # Stage DRAM->SBUF keeping contiguous f-runs innermost
                # (DMA needs a shared contiguous final dim, <=3 dims); the
                # transpose into per-bin windows happens on vector engine.
                xst = spool.tile([C, 2 * B * NFB], dt.bfloat16, tag="xst")
                xsr = xst.rearrange("c (b f) -> c b f", f=NFB)
                nc.gpsimd.dma_start(
                    out=xsr[:, 0:B], in_=xre[:, :, fsl].rearrange("b c f -> c b f")
                )
                nc.gpsimd.dma_start(
                    out=xsr[:, B : 2 * B],
                    in_=xim[:, :, fsl].rearrange("b c f -> c b f"),
                )
                wstp = spool.tile([C, O * NFB], dt.bfloat16, tag="wstp")
                wstq = spool.tile([C, O * NFB], dt.bfloat16, tag="wstq")
                wpr = wstp.rearrange("c (o f) -> c o f", f=NFB)
                wqr = wstq.rearrange("c (o f) -> c o f", f=NFB)
                for k in range(NOCH):
                    osl = slice(OC * k, OC * (k + 1))
                    nc.gpsimd.dma_start(
                        out=wpr[:, osl],
                        in_=wim[k][:, :, fsl].rearrange("o c f -> c o f"),
                    )
                    nc.gpsimd.dma_start(
                        out=wqr[:, osl],
                        in_=wre[k][:, :, fsl].rearrange("o c f -> c o f"),
                    )

                xt = xpool.tile([C, NFB * WIN], dt.bfloat16, tag="x")
                xtr = xt.rearrange("c (f z) -> c f z", f=NFB)
                # window per bin: [A 0:32 | B 32:64 | P 64:192 | Q 192:320 | Pn 320:448]
                nc.vector.tensor_copy(
                    xtr[:, :, 0 : 2 * B], xst.rearrange("c (b f) -> c f b", f=NFB)
                )
                nc.vector.tensor_copy(
                    xtr[:, :, 64:192], wstp.rearrange("c (o f) -> c f o", f=NFB)
                )
                nc.vector.tensor_copy(
                    xtr[:, :, 192:320], wstq.rearrange("c (o f) -> c f o", f=NFB)
                )
                nc.vector.tensor_scalar_mul(
                    xtr[:, :, 320:448],
                    wstp.rearrange("c (o f) -> c f o", f=NFB),
                    -1.0,
                )

                # yo col = z*NFB + f  (z = r*O + o), so f stays contiguous
                yo = ypool.tile([B, 2 * O * NFB], dt.bfloat16, tag="yo")
                yor = yo.rearrange("b (z f) -> b z f", f=NFB)
                for p in range(0, NFB, 2):
                    gn = min(2, NFB - p)
                    ps = pspool.tile([B, gn * 256], dt.float32, tag="ps")
                    for j in range(gn):
                        wb = (p + j) * WIN
                        sl = slice(j * 256, (j + 1) * 256)
                        # (A.Q | -A.P) + (B.P | B.Q) -> (yr | yi)
                        nc.tensor.matmul(
                            ps[:, sl], xt[:, wb : wb + 32],
                            xt[:, wb + 192 : wb + 448], start=True, stop=False,
                        )
                        nc.tensor.matmul(
                            ps[:, sl], xt[:, wb + 32 : wb + 64],
                            xt[:, wb + 64 : wb + 320], start=False, stop=True,
                        )
                    nc.vector.tensor_copy(
                        yor[:, :, p : p + gn],
                        ps.rearrange("b (f z) -> b z f", f=gn),
                    )
                nc.gpsimd.dma_start(
                    out=y[:, :, fsl], in_=yor
                )
    nc.compile()
    return nc


def _make_runner(nc):
    """Vendored from bass2jax.run_bass_via_pjrt: same custom-call path, but
    accepts pre-committed sharded device arrays (so uploads overlap host
    work) and returns the device output array without blocking."""
    import jax
    from jax.sharding import Mesh, PartitionSpec
    from jax.experimental.shard_map import shard_map
    from concourse import bass2jax, mybir

    bass2jax.install_neuronx_cc_hook()

    partition_name = nc.partition_id_tensor.name if nc.partition_id_tensor else None
    in_names, out_names, out_avals, out_shapes = [], [], [], []
    for alloc in nc.m.functions[0].allocations:
        if type(alloc).__name__ != "MemoryLocationSet":
            continue
        name = alloc.memorylocations[0].name
        if alloc.kind == "ExternalInput":
            if name != partition_name:
                in_names.append(name)
        elif alloc.kind == "ExternalOutput":
            shape = tuple(alloc.tensor_shape)
            dtype = mybir.dt.np(alloc.dtype)
            out_names.append(name)
            out_avals.append(jax.core.ShapedArray(shape, dtype))
            out_shapes.append((shape, dtype))
    n_params = len(in_names)
    all_names = in_names + out_names
    if partition_name is not None:
        all_names = all_names + [partition_name]
    donate = tuple(range(n_params, n_params + len(out_names)))

    def _body(*args):
        operands = list(args)
        if partition_name is not None:
            operands.append(bass2jax.partition_id_tensor())
        outs = bass2jax._bass_exec_p.bind(
            *operands,
            out_avals=tuple(out_avals),
            in_names=tuple(all_names),
            out_names=tuple(out_names),
            lowering_input_output_aliases=(),
            sim_require_finite=True,
            sim_require_nnan=True,
            nc=nc,
        )
        return tuple(outs)

    devices = jax.devices()[:NCORES]
    mesh = Mesh(np.asarray(devices), ("core",))
    nargs = n_params + len(out_names)
    sharded = jax.jit(
        shard_map(
            _body,
            mesh=mesh,
            in_specs=(PartitionSpec("core"),) * nargs,
            out_specs=(PartitionSpec("core"),) * len(out_names),
            check_rep=False,
        ),
        donate_argnums=donate,
        keep_unused=True,
    )
    return sharded, in_names, out_names, out_shapes, mesh


def kernel(x: np.ndarray, weight: np.ndarray, bias: np.ndarray) -> np.ndarray:
    import threading
    import ml_dtypes
    import scipy.fft as sf
    import jax

    bf16 = ml_dtypes.bfloat16
    x = np.asarray(x, np.float32)
    weight = np.asarray(weight, np.float32)
    bias = np.asarray(bias, np.float32)

    # The axon device handshake takes ~1s of pure I/O; overlap it with the
    # first FFT chunk.
    if "mesh" not in _cache:
        ready = threading.Event()

        def _init():
            from jax.sharding import Mesh

            devices = jax.devices()[:NCORES]
            _cache["mesh"] = Mesh(np.asarray(devices), ("core",))
            ready.set()

        threading.Thread(target=_init, daemon=True).start()
    else:
        ready = None

    from jax.sharding import NamedSharding, PartitionSpec

    puts = {}
    pending = []

    def put(name, arr):
        puts[name] = jax.device_put(arr, _cache["sharding"])
        pending.append(puts[name])

    # --- weight spectrum, chunked over o so cast overlaps upload ---
    OC = O // NOCH
    for k in range(NOCH):
        wf = sf.rfft(weight[k * OC : (k + 1) * OC], n=N, axis=-1)  # [OC,C,F] c64
        g = np.zeros((NCORES * OC, 2, C, FC), bf16)
        for r in range(NCORES):
            sl = slice(r * FC, min((r + 1) * FC, F))
            n = sl.stop - sl.start
            g[OC * r : OC * r + OC, 0, :, :n] = wf.imag[:, :, sl].astype(bf16)
            g[OC * r : OC * r + OC, 1, :, :n] = wf.real[:, :, sl].astype(bf16)
        if ready is not None and k == 0:
            ready.wait()
            _cache["sharding"] = NamedSharding(
                _cache["mesh"], PartitionSpec("core")
            )
        put(f"wc{k}", g)

    # --- x spectrum ---
    xf = sf.rfft(x, n=N, axis=-1)  # [B,C,F] c64
    gx = np.zeros((NCORES * B, 2, C, FC), bf16)
    for r in range(NCORES):
        sl = slice(r * FC, min((r + 1) * FC, F))
        n = sl.stop - sl.start
        gx[B * r : B * r + B, 0, :, :n] = xf.real[:, :, sl].astype(bf16)
        gx[B * r : B * r + B, 1, :, :n] = xf.imag[:, :, sl].astype(bf16)
    put("xc", gx)

    # donated zero output buffers (zeros compress well on the tunnel)
    if "nc" not in _cache:
        _cache["nc"] = _build_bass()
    zero_names = ["y"]
    zero_args = [
        jax.device_put(np.zeros((NCORES * B, 2 * O, FC), bf16), _cache["sharding"])
    ]

    # --- jit build/compile while uploads stream ---
    if "runner" not in _cache:
        _cache["runner"] = _make_runner(_cache["nc"])
    sharded, in_names, out_names, out_shapes, mesh = _cache["runner"]

    args = [puts[name] for name in in_names] + zero_args
    out_arrs = sharded(*args)
    yg = np.asarray(out_arrs[0])  # [8*B, 2*O, FC] bf16

    # --- host: assemble spectrum, twiddle, bias, inverse rfft ---
    ygf = yg.astype(np.float32).reshape(NCORES, B, 2, O, FC)
    yc = np.empty((B, O, NCORES * FC), np.complex64)
    for r in range(NCORES):
        yc.real[:, :, FC * r : FC * (r + 1)] = ygf[r, :, 0]
        yc.imag[:, :, FC * r : FC * (r + 1)] = ygf[r, :, 1]
    yv = yc[:, :, :F]
    # fold out the causal left-pad shifts: x by K-1=4096 -> (-1)^f, w by 1
    tw = np.exp(1j * np.pi * np.arange(F) * (N // 2 + 1) / (N // 2)).astype(
        np.complex64
    )
    yv *= tw
    yv[:, :, 0] += (bias * np.float32(N)).astype(np.float32)[None, :]
    out = sf.irfft(yv, n=N, axis=-1)[:, :, :L]
    return np.ascontiguousarray(out, dtype=np.float32)
